# revision 13
# baseline (speedup 1.0000x reference)
"""GQA causal attention (B=2,S=2048,D=2048,H=16,KV=4,HD=128) on 8 TRN2 NeuronCores.

Sharding: core c handles (batch b=c//4, kv-group g=c%4) — exactly 8 shards.
Each core computes q/k/v projections for its group's 4 query heads + 1 kv head,
RoPE, causal attention (512-wide q tiles, skipping fully-masked k blocks),
and a partial o-projection over its heads' slice of wo. Host sums the 4
group-partials per batch.

v2 (this file) vs the f32r baseline:
  - whole matmul datapath in bf16 (x, wq/wk/wv/wo, qT/kT/vn, e, outT, out).
    PSUM accumulation stays f32; rope/softmax-normalize math stays f32.
    Halves the input DMA (x was the phase-1 bottleneck: it rode a ~100GB/s
    SWDGE queue while total SDMA is ~350GB/s shared) and lifts the f32r
    N>=256 restriction so diagonal score blocks narrow to their live range.
  - all bulk DMA on the two HWDGE rings (sync + scalar); gpsimd SWDGE only
    for tiny loads. x on sync, weights on scalar, first tile split fine.
  - phase 2 restructured h-inner: per (t, j) the 4 heads' scores/exp/AV run
    back-to-back and the 4 Z (softmax denominator) matmuls are col-tiled
    (M=1 at tile_position (0,32h)) into one PSUM tile so they overlap on
    distinct PE column groups instead of costing a full third pass.
  - o-projection moved after the attention loop (attention needs all 8 PSUM
    banks: 3 score + 4 AV + 1 Z); its rows are ordered so earlier q-tiles'
    output flows while the last tile's normalize chain drains.

Device layouts are all "transposed" ([feature, seq]) so no on-device
transposes of activations are needed; head-dim is PERMUTED to [evens|odds]
(folded into wq/wk columns host-side) so RoPE is two partition-aligned
half-tile multiplies.
"""

import numpy as np

B, S, D = 2, 2048, 2048
H, KV, HD = 16, 4, 128
GH = H // KV            # query heads per kv group (per core)
NCORES = 8
THETA = 10000.0
NEG = -1e9
SQT = 512               # q seq tile width
NSQ = S // SQT          # 4
NKB = S // 128          # 16 k blocks
NCH = D // 128          # 16 contraction chunks
NOFF = SQT // 128       # 4 diagonal offsets

SCALE = float(HD) ** -0.5

# Diagonal-region block at offset o is causally dead below sq_local = 128*o;
# bf16 matmuls run full rate at any N, so narrow exactly to the live range.
NARROW = [0, 128, 256, 384]
# The 128-wide triangular column range needing the 0/1 mask multiply.
MASKW = [(0, 128), (128, 256), (256, 384), (384, 512)]

# Exposed for the dev harness (test.py) to read profiling results.
last_results = None


def _build_program():
    from contextlib import ExitStack

    import concourse.tile as tile
    from concourse import bacc, mybir
    from concourse.masks import make_identity

    f32 = mybir.dt.float32
    bf16 = mybir.dt.bfloat16
    EXP = mybir.ActivationFunctionType.Exp

    nc = bacc.Bacc("TRN2", target_bir_lowering=False, debug=False,
                   num_devices=NCORES)

    # all bulk tensors are pre-rearranged on the host so every DMA is
    # contiguous per partition
    xT_d = nc.dram_tensor("xr", [128, NSQ, NCH, SQT], bf16, kind="ExternalInput")
    wq_d = nc.dram_tensor("wqp", [128, NCH, GH * HD], bf16, kind="ExternalInput")
    wk_d = nc.dram_tensor("wkp", [128, NCH, HD], bf16, kind="ExternalInput")
    wv_d = nc.dram_tensor("wvg", [128, NCH, HD], bf16, kind="ExternalInput")
    wo_d = nc.dram_tensor("wog", [128, GH, D], bf16, kind="ExternalInput")
    cos_d = nc.dram_tensor("cos2", [HD, S], f32, kind="ExternalInput")
    sin_d = nc.dram_tensor("sinS", [HD, S], f32, kind="ExternalInput")
    msk_d = nc.dram_tensor("m01", [128, NOFF, SQT], bf16, kind="ExternalInput")
    one_d = nc.dram_tensor("ones1", [128, 1], bf16, kind="ExternalInput")
    out_d = nc.dram_tensor("out", [128, S // 128, D], bf16, kind="ExternalOutput")

    xT_v = xT_d.ap()        # [128, NSQ, NCH, SQT]
    wq_v = wq_d.ap()
    wk_v = wk_d.ap()
    wv_v = wv_d.ap()
    wo_v = wo_d.ap()
    out_v = out_d.ap()      # [128, 16, 2048]; host untangles (m p) rows

    with tile.TileContext(nc) as tc, ExitStack() as ctx:
        persist = ctx.enter_context(tc.tile_pool(name="persist", bufs=1))

        qT = [persist.tile([128, S], bf16, name=f"qT{h}") for h in range(GH)]
        kT = persist.tile([128, S], bf16, name="kT")
        vn = persist.tile([128, NKB, HD], bf16, name="vn")
        cos2 = persist.tile([128, S], f32, name="cos2")
        sinS = persist.tile([128, S], f32, name="sinS")
        m01 = persist.tile([128, NOFF, SQT], bf16, name="m01")
        ones = persist.tile([128, 1], bf16, name="ones")
        ident = persist.tile([128, 128], bf16, name="ident")

        nc.gpsimd.dma_start(ones[:], one_d[:])
        make_identity(nc, ident[:])
        # dummy broadcast: loads the gpsimd ucode overlay (~10us) off the
        # critical path — the first real one otherwise stalls t=0 normalize
        warm = persist.tile([128, 1], bf16, name="warm")
        nc.gpsimd.partition_broadcast(warm[:], ones[0:1, :])

        # ---------------- Phase 1: projections + RoPE + v ----------------
        with (
            tc.tile_pool(name="w1", bufs=1) as w1p,
            tc.tile_pool(name="xa", bufs=2) as xap,
            tc.tile_pool(name="raw", bufs=2) as rawp,
            tc.tile_pool(name="rope", bufs=2) as ropep,
            tc.tile_pool(name="ps1", bufs=1, space="PSUM") as ps1,
            tc.tile_pool(name="tps", bufs=2, space="PSUM") as tps,
        ):
            # wq and t0's x in 2-chunk piece TILES: cross-engine waits are
            # tile-granular, so small tiles let the first matmuls fire as
            # soon as their own 0.25MB lands instead of after the whole
            # tensor. All three rings split ~310GB/s early — keep the
            # first-15us traffic to exactly what the PE needs.
            wq_p8 = [w1p.tile([128, 2, GH * HD], bf16, name=f"wq{i}")
                     for i in range(8)]
            wk_sb = w1p.tile([128, NCH, HD], bf16)
            wv_sb = w1p.tile([128, NCH, HD], bf16)
            vT_tmp = w1p.tile([128, S], bf16)
            x0_p8 = [xap.tile([128, 2, SQT], bf16, tag=f"x0_{i}",
                              name=f"x0_{i}") for i in range(8)]
            for i in range(8):
                nc.scalar.dma_start(wq_p8[i][:], wq_v[:, 2 * i:2 * i + 2, :])
                nc.sync.dma_start(x0_p8[i][:], xT_v[:, 0, 2 * i:2 * i + 2, :])
            nc.scalar.dma_start(wk_sb[:], wk_v)
            nc.scalar.dma_start(wv_sb[:], wv_v)
            # rope/mask aux on the (otherwise idle) SWDGE ring
            nc.gpsimd.dma_start(cos2[:], cos_d[:])
            nc.gpsimd.dma_start(sinS[:], sin_d[:])
            nc.gpsimd.dma_start(m01[:], msk_d[:])

            def rope(raw, dst, t):
                """dst[:, t-tile] = rope(raw) in the [evens|odds] layout."""
                sl = np.s_[:, t * SQT:(t + 1) * SQT]
                tmp = ropep.tile([128, SQT], f32, tag="ropetmp", name="tmp")
                swp = ropep.tile([128, SQT], f32, tag="ropeswp", name="swp")
                nc.vector.tensor_mul(tmp[:], raw[:], cos2[sl])
                # swp[0:64] = odd*(-sin), swp[64:128] = even*(+sin); sinS is
                # stored [+sin | -sin] so each mul's two INPUTS share a base
                # partition (walrus requires that); only the output crosses.
                nc.vector.tensor_mul(swp[0:64, :], raw[64:128, :],
                                     sinS[sl][64:128, :])
                nc.vector.tensor_mul(swp[64:128, :], raw[0:64, :],
                                     sinS[sl][0:64, :])
                nc.vector.tensor_add(dst[sl], tmp[:], swp[:])

            for t in range(NSQ):
                ssl = np.s_[t * SQT:(t + 1) * SQT]
                q_ps = [ps1.tile([128, SQT], f32, tag=f"qps{h}", name=f"qps{h}")
                        for h in range(GH)]
                k_ps = ps1.tile([128, SQT], f32, tag="kps", name="k_ps")
                v_ps = ps1.tile([128, SQT], f32, tag="vps", name="v_ps")
                if t == 0:
                    def xc(c):
                        return x0_p8[c // 2][:, c % 2, :]
                else:
                    # one full-tile DMA: 16KB-per-partition descriptors give
                    # the HWDGE ring ~2x the throughput of 8KB ones
                    xt = xap.tile([128, NCH, SQT], bf16, tag="xtf",
                                  name="xtf")
                    nc.sync.dma_start(xt[:], xT_v[:, t, :, :])

                    def xc(c):
                        return xt[:, c, :]
                # all q matmuls first, then k/v: the PE stream is in-order,
                # and the (later-arriving) wk/wv DMAs must not stall it
                # while q chunks are ready
                for c in range(NCH):
                    st, sp = c == 0, c == NCH - 1
                    for h in range(GH):
                        nc.tensor.matmul(
                            q_ps[h][:],
                            wq_p8[c // 2][:, c % 2, h * HD:(h + 1) * HD],
                            xc(c), start=st, stop=sp)
                for c in range(NCH):
                    st, sp = c == 0, c == NCH - 1
                    nc.tensor.matmul(k_ps[:], wk_sb[:, c, :],
                                     xc(c), start=st, stop=sp)
                    nc.tensor.matmul(v_ps[:], wv_sb[:, c, :],
                                     xc(c), start=st, stop=sp)
                # psum -> sbuf copies split over ACT/DVE; q0/q1 drain FIRST
                # so the next t's leading matmuls get their banks back,
                # transposes before rope so the DVE queue doesn't block the
                # PE on freeing transpose psum slots
                qraws = []
                for h in range(GH):
                    qraw = rawp.tile([128, SQT], f32, tag=f"qraw{h}",
                                     name=f"qraw{h}")
                    qraws.append(qraw)
                nc.scalar.copy(qraws[0][:], q_ps[0][:])
                nc.vector.tensor_copy(qraws[1][:], q_ps[1][:])
                nc.scalar.copy(vT_tmp[:, ssl], v_ps[:])
                for j in range(NOFF * t, NOFF * (t + 1)):
                    t_ps = tps.tile([128, 128], bf16, tag="tps", name="t_ps")
                    nc.tensor.transpose(
                        t_ps[:], vT_tmp[:, j * 128:(j + 1) * 128], ident[:])
                    if j % 2 == 0:
                        nc.vector.tensor_copy(vn[:, j, :], t_ps[:])
                    else:
                        nc.scalar.copy(vn[:, j, :], t_ps[:])
                kraw = rawp.tile([128, SQT], f32, tag="kraw", name="kraw")
                nc.scalar.copy(kraw[:], k_ps[:])
                rope(kraw, kT, t)
                for h in range(GH):
                    if h == 2:
                        nc.scalar.copy(qraws[2][:], q_ps[2][:])
                    elif h == 3:
                        nc.vector.tensor_copy(qraws[3][:], q_ps[3][:])
                    rope(qraws[h], qT[h], t)

        # -------- Phase 2: attention (h-inner), then o-projection --------
        with (
            tc.tile_pool(name="wo", bufs=1) as wop,
            tc.tile_pool(name="ot", bufs=1) as otp,
            tc.tile_pool(name="ex", bufs=2) as exp_p,
            tc.tile_pool(name="nrm", bufs=2) as nrm_p,
            tc.tile_pool(name="osb", bufs=2) as osb_p,
        ):
            wo_sb = wop.tile([128, GH, D], bf16)
            nc.scalar.dma_start(wo_sb[:], wo_v)
            outT = [otp.tile([128, S], bf16, name=f"outT{h}")
                    for h in range(GH)]

            # o-projection chunks: one (m, jd) chunk = 4 accumulating
            # matmuls + a psum copy. Chunks are woven INTO the attention
            # j-loop as PE filler: attention alone is exp(ACT)-paced, and a
            # PE that micro-idles gets clocked down (p-state/HAM) to half
            # rate — the filler keeps it dense and at full clock.
            ob_map = {}
            ochunks = []        # ready (m, jd) chunks, global order
            oc_i = 0

            def o_chunk(ops_pool):
                nonlocal oc_i
                m, jd = ochunks[oc_i]
                oc_i += 1
                if jd == 0:
                    ob_map[m] = osb_p.tile([128, D], bf16, tag="ob",
                                           name="ob")
                dsl = np.s_[jd * SQT:(jd + 1) * SQT]
                o_ps = ops_pool.tile([128, SQT], f32, tag="o", name="o_ps")
                for h in range(GH):
                    nc.tensor.matmul(
                        o_ps[:], outT[h][:, m * 128:(m + 1) * 128],
                        wo_sb[:, h, dsl],
                        start=(h == 0), stop=(h == GH - 1))
                if (m + jd) % 2 == 0:
                    nc.vector.tensor_copy(ob_map[m][:, dsl], o_ps[:])
                else:
                    nc.scalar.copy(ob_map[m][:, dsl], o_ps[:])
                # store per-jd so the final rows drain while the PE still
                # computes (shrinks the end-of-kernel DMA tail)
                if (m + jd) % 2 == 0:
                    nc.sync.dma_start(out_v[:, m, dsl], ob_map[m][:, dsl])
                else:
                    nc.scalar.dma_start(out_v[:, m, dsl], ob_map[m][:, dsl])

            # t=1 first: its leading blocks are off-diagonal (no DVE mask in
            # the chain), so attention streams while DVE drains the phase-1
            # rope backlog; all-diagonal t=0 follows.
            with (
                tc.tile_pool(name="sps", bufs=2, space="PSUM") as sps_p,
                tc.tile_pool(name="avps", bufs=1, space="PSUM") as avp_p,
                tc.tile_pool(name="zps", bufs=1, space="PSUM") as zp_p,
                tc.tile_pool(name="opsi", bufs=1, space="PSUM") as ops_i,
            ):
                for t in [1, 0, 2, 3]:
                    qsl = np.s_[:, t * SQT:(t + 1) * SQT]
                    nblk = NOFF * (t + 1)
                    av_ps = [avp_p.tile([HD, SQT], f32, tag=f"av{h}",
                                        name=f"av{h}") for h in range(GH)]
                    z4 = zp_p.tile([128, SQT], f32, tag="z4", name="z4")
                    for j in range(nblk):
                        o = j - NOFF * t
                        lo = NARROW[o] if o >= 0 else 0
                        csl = np.s_[:, lo:SQT]
                        st, sp = j == 0, j == nblk - 1
                        # scores for all 4 heads first (s0,s1 | filler |
                        # s2,s3): by the time av_h streams, exp_h is done —
                        # the PE never blocks on the ACT engine
                        s_tiles = []
                        for h in range(GH):
                            s_ps = sps_p.tile([128, SQT], f32, tag="s",
                                              name="s_ps")
                            nc.tensor.matmul(
                                s_ps[csl], kT[:, j * 128:(j + 1) * 128],
                                qT[h][qsl][csl])
                            s_tiles.append(s_ps)
                            if h == 1 and oc_i < len(ochunks):
                                o_chunk(ops_i)
                        es = []
                        for h in range(GH):
                            e = exp_p.tile([128, SQT], bf16, tag=f"e{h}",
                                           name=f"e{h}")
                            nc.scalar.activation(e[csl], s_tiles[h][csl],
                                                 EXP, scale=SCALE)
                            if o >= 0:  # mask the causally-dead triangle
                                ma, mb = MASKW[o]
                                msl = np.s_[:, ma:mb]
                                nc.vector.tensor_mul(e[msl], e[msl],
                                                     m01[:, o, ma:mb])
                            es.append(e)
                        for h in range(GH):
                            nc.tensor.matmul(av_ps[h][csl], vn[:, j, :],
                                             es[h][csl], start=st, stop=sp)
                        # Z (softmax denominator) for all 4 heads: M=1
                        # matmuls col-tiled to distinct PE column groups so
                        # they overlap; head h's row lives at partition 32h
                        # (engine APs may only start at partitions 0/32/64/96)
                        for h in range(GH):
                            nc.tensor.matmul(
                                z4[32 * h:32 * h + 1, lo:SQT], ones[:],
                                es[h][csl], start=st, stop=sp,
                                tile_position=(0, 32 * h))
                    # free the AV psum banks fast: unnormalized copy on ACT
                    av_sb = []
                    for h in range(GH):
                        avo = nrm_p.tile([HD, SQT], f32, tag=f"avo{h}",
                                         name=f"avo{h}", bufs=1)
                        nc.scalar.copy(avo[:], av_ps[h][:])
                        av_sb.append(avo)
                    # unused zall rows memset to 1.0 so the batched
                    # reciprocal stays finite
                    zall = nrm_p.tile([128, SQT], f32, tag="zall", name="zall")
                    nc.gpsimd.memset(zall[:], 1.0)
                    for h in range(GH):
                        nc.vector.tensor_copy(zall[32 * h:32 * h + 1, :],
                                              z4[32 * h:32 * h + 1, :])
                    zrec = nrm_p.tile([128, SQT], f32, tag="zrec", name="zrec")
                    # ~18 significant bits — plenty above bf16 data, 5x
                    # faster than exact reciprocal; zall has no zeros/infs
                    nc.vector.reciprocal_approx_fast(zrec[:], zall[:])
                    for h in range(GH):
                        # stage to base partition 0: the HW broadcast ucode
                        # does not honor a nonzero AP base partition
                        z1 = nrm_p.tile([1, SQT], f32, tag="z1", name="z1")
                        nc.vector.tensor_copy(z1[:], zrec[32 * h:32 * h + 1, :])
                        zb = nrm_p.tile([128, SQT], f32, tag="zb", name="zb")
                        nc.gpsimd.partition_broadcast(zb[:], z1[:])
                        nc.vector.tensor_mul(outT[h][qsl], av_sb[h][:], zb[:])
                    # this tile's o-proj chunks become available as filler
                    # for the NEXT tile's attention
                    ochunks.extend([(m, jd) for m in
                                    range(NOFF * t, NOFF * (t + 1))
                                    for jd in range(D // SQT)])

            # remaining o-projection chunks back-to-back (deeper psum
            # rotation so copies never gate the PE)
            with tc.tile_pool(name="ops", bufs=4, space="PSUM") as ops_t:
                while oc_i < len(ochunks):
                    o_chunk(ops_t)

    nc.compile()
    return nc


_prog = None


def _host_inputs(x, wq, wk, wv, wo):
    """Per-core input maps (core c -> batch c//KV, kv-group c%KV)."""
    import ml_dtypes
    bf16 = ml_dtypes.bfloat16

    perm = np.concatenate([np.arange(0, HD, 2), np.arange(1, HD, 2)])
    wq_p = np.ascontiguousarray(
        wq.reshape(D, H, HD)[:, :, perm].reshape(D, H * HD))
    wk_p = np.ascontiguousarray(
        wk.reshape(D, KV, HD)[:, :, perm].reshape(D, KV * HD))

    inv_freq = 1.0 / (THETA ** (np.arange(0, HD, 2, dtype=np.float64) / HD))
    freqs = np.outer(np.arange(S, dtype=np.float64), inv_freq)   # [S, 64]
    cosT = np.cos(freqs).T.astype(np.float32)                    # [64, S]
    sinT = np.sin(freqs).T.astype(np.float32)
    cos2 = np.ascontiguousarray(np.concatenate([cosT, cosT], 0))
    sinS = np.ascontiguousarray(np.concatenate([sinT, -sinT], 0))

    sk = np.arange(128)[:, None]
    sq = np.arange(SQT)[None, :]
    m01 = np.stack([(sk <= sq - 128 * o).astype(np.float32)
                    for o in range(NOFF)], axis=1)               # [128,4,512]
    m01 = np.ascontiguousarray(m01).astype(bf16)

    def part_major(a):
        """[NCH*128, W] -> [128, NCH, W] (chunk-of-contraction per partition)."""
        w = a.shape[1]
        return np.ascontiguousarray(
            a.reshape(NCH, 128, w).transpose(1, 0, 2)).astype(bf16)

    in_maps = []
    for c in range(NCORES):
        b, g = c // KV, c % KV
        xr = np.ascontiguousarray(
            x[b].T.reshape(NCH, 128, NSQ, SQT).transpose(1, 2, 0, 3)
        ).astype(bf16)
        wog = np.ascontiguousarray(
            wo[g * GH * HD:(g + 1) * GH * HD, :].reshape(GH, 128, D)
            .transpose(1, 0, 2)).astype(bf16)
        in_maps.append({
            "xr": xr,
            "wqp": part_major(wq_p[:, g * GH * HD:(g + 1) * GH * HD]),
            "wkp": part_major(wk_p[:, g * HD:(g + 1) * HD]),
            "wvg": part_major(wv[:, g * HD:(g + 1) * HD]),
            "wog": wog,
            "cos2": cos2,
            "sinS": sinS,
            "m01": m01,
            "ones1": np.ones((128, 1), bf16),
        })
    return in_maps


def _numpy_reference(x, mask, wq, wk, wv, wo):
    """Pure-numpy fallback for inputs this kernel isn't specialized for."""
    b, s, _ = x.shape
    q = (x @ wq).reshape(b, s, H, HD)
    k = (x @ wk).reshape(b, s, KV, HD)
    v = (x @ wv).reshape(b, s, KV, HD)
    inv_freq = 1.0 / (THETA ** (np.arange(0, HD, 2, dtype=np.float32) / HD))
    t = np.arange(s, dtype=np.float32)
    freqs = np.outer(t, inv_freq)
    cos = np.cos(freqs)[:, None, :]
    sin = np.sin(freqs)[:, None, :]

    def rot(a):
        bb, ss, nh, hd = a.shape
        a = a.reshape(bb, ss, nh, hd // 2, 2)
        a0, a1 = a[..., 0], a[..., 1]
        out = np.stack([a0 * cos - a1 * sin, a0 * sin + a1 * cos], axis=-1)
        return out.reshape(bb, ss, nh, hd)

    q, k = rot(q), rot(k)
    rep = H // KV
    k = np.repeat(k, rep, axis=2)
    v = np.repeat(v, rep, axis=2)
    q, k, v = (a.transpose(0, 2, 1, 3) for a in (q, k, v))
    scores = np.einsum("bhqd,bhkd->bhqk", q, k) * SCALE + mask
    scores = scores - scores.max(axis=-1, keepdims=True)
    e = np.exp(scores)
    attn = e / e.sum(axis=-1, keepdims=True)
    out = np.einsum("bhqk,bhkd->bhqd", attn, v)
    out = out.transpose(0, 2, 1, 3).reshape(b, s, H * HD)
    return (out @ wo).astype(np.float32)


def kernel(x, mask, wq, wk, wv, wo):
    global _prog, last_results
    x = np.asarray(x, np.float32)
    mask = np.asarray(mask, np.float32)
    wq, wk, wv, wo = (np.asarray(a, np.float32) for a in (wq, wk, wv, wo))

    causal = np.where(np.tril(np.ones((S, S), bool)), 0.0, NEG).astype(np.float32)
    if (x.shape != (B, S, D) or mask.shape != (S, S)
            or not np.array_equal(mask, causal)):
        return _numpy_reference(x, mask, wq, wk, wv, wo)

    from concourse import bass_utils

    if _prog is None:
        _prog = _build_program()

    in_maps = _host_inputs(x, wq, wk, wv, wo)
    last_results = bass_utils.run_bass_kernel_spmd(
        _prog, in_maps, core_ids=list(range(NCORES)))
    # device layout [128, 16, D]: logical row = m*128 + p
    parts = [res["out"].astype(np.float32).transpose(1, 0, 2).reshape(S, D)
             for res in last_results.results]
    out = np.empty((B, S, D), np.float32)
    for b in range(B):
        out[b] = parts[KV * b] + parts[KV * b + 1] + parts[KV * b + 2] + parts[KV * b + 3]
    return out


# revision 15
# speedup vs baseline: 1.0240x; 1.0240x over previous
"""GQA causal attention (B=2,S=2048,D=2048,H=16,KV=4,HD=128) on 8 TRN2 NeuronCores.

Sharding: core c handles (batch b=c//4, kv-group g=c%4) — exactly 8 shards.
Each core computes q/k/v projections for its group's 4 query heads + 1 kv head,
RoPE, causal attention (512-wide q tiles, skipping fully-masked k blocks),
and a partial o-projection over its heads' slice of wo. Host sums the 4
group-partials per batch.

v2 (this file) vs the f32r baseline:
  - whole matmul datapath in bf16 (x, wq/wk/wv/wo, qT/kT/vn, e, outT, out).
    PSUM accumulation stays f32; rope/softmax-normalize math stays f32.
    Halves the input DMA (x was the phase-1 bottleneck: it rode a ~100GB/s
    SWDGE queue while total SDMA is ~350GB/s shared) and lifts the f32r
    N>=256 restriction so diagonal score blocks narrow to their live range.
  - all bulk DMA on the two HWDGE rings (sync + scalar); gpsimd SWDGE only
    for tiny loads. x on sync, weights on scalar, first tile split fine.
  - phase 2 restructured h-inner: per (t, j) the 4 heads' scores/exp/AV run
    back-to-back and the 4 Z (softmax denominator) matmuls are col-tiled
    (M=1 at tile_position (0,32h)) into one PSUM tile so they overlap on
    distinct PE column groups instead of costing a full third pass.
  - o-projection moved after the attention loop (attention needs all 8 PSUM
    banks: 3 score + 4 AV + 1 Z); its rows are ordered so earlier q-tiles'
    output flows while the last tile's normalize chain drains.

Device layouts are all "transposed" ([feature, seq]) so no on-device
transposes of activations are needed; head-dim is PERMUTED to [evens|odds]
(folded into wq/wk columns host-side) so RoPE is two partition-aligned
half-tile multiplies.
"""

import numpy as np

B, S, D = 2, 2048, 2048
H, KV, HD = 16, 4, 128
GH = H // KV            # query heads per kv group (per core)
NCORES = 8
THETA = 10000.0
NEG = -1e9
SQT = 512               # q seq tile width
NSQ = S // SQT          # 4
NKB = S // 128          # 16 k blocks
NCH = D // 128          # 16 contraction chunks
NOFF = SQT // 128       # 4 diagonal offsets

SCALE = float(HD) ** -0.5

# Diagonal-region block at offset o is causally dead below sq_local = 128*o;
# bf16 matmuls run full rate at any N, so narrow exactly to the live range.
NARROW = [0, 128, 256, 384]
# The 128-wide triangular column range needing the 0/1 mask multiply.
MASKW = [(0, 128), (128, 256), (256, 384), (384, 512)]

# Exposed for the dev harness (test.py) to read profiling results.
last_results = None


def _build_program():
    from contextlib import ExitStack

    import concourse.tile as tile
    from concourse import bacc, mybir
    from concourse.masks import make_identity

    f32 = mybir.dt.float32
    bf16 = mybir.dt.bfloat16
    EXP = mybir.ActivationFunctionType.Exp

    nc = bacc.Bacc("TRN2", target_bir_lowering=False, debug=False,
                   num_devices=NCORES)

    # all bulk tensors are pre-rearranged on the host so every DMA is
    # contiguous per partition
    xT_d = nc.dram_tensor("xr", [128, NSQ, NCH, SQT], bf16, kind="ExternalInput")
    wq_d = nc.dram_tensor("wqp", [128, NCH, GH * HD], bf16, kind="ExternalInput")
    wk_d = nc.dram_tensor("wkp", [128, NCH, HD], bf16, kind="ExternalInput")
    wv_d = nc.dram_tensor("wvg", [128, NCH, HD], bf16, kind="ExternalInput")
    wo_d = nc.dram_tensor("wog", [128, GH, D], bf16, kind="ExternalInput")
    cos_d = nc.dram_tensor("cos2", [HD, S], f32, kind="ExternalInput")
    sin_d = nc.dram_tensor("sinS", [HD, S], f32, kind="ExternalInput")
    msk_d = nc.dram_tensor("m01", [128, NOFF, SQT], bf16, kind="ExternalInput")
    one_d = nc.dram_tensor("ones1", [128, 1], bf16, kind="ExternalInput")
    out_d = nc.dram_tensor("out", [128, S // 128, D], bf16, kind="ExternalOutput")

    xT_v = xT_d.ap()        # [128, NSQ, NCH, SQT]
    wq_v = wq_d.ap()
    wk_v = wk_d.ap()
    wv_v = wv_d.ap()
    wo_v = wo_d.ap()
    out_v = out_d.ap()      # [128, 16, 2048]; host untangles (m p) rows

    with tile.TileContext(nc) as tc, ExitStack() as ctx:
        persist = ctx.enter_context(tc.tile_pool(name="persist", bufs=1))

        qT = [persist.tile([128, S], bf16, name=f"qT{h}") for h in range(GH)]
        kT = persist.tile([128, S], bf16, name="kT")
        vn = persist.tile([128, NKB, HD], bf16, name="vn")
        cos2 = persist.tile([128, S], f32, name="cos2")
        sinS = persist.tile([128, S], f32, name="sinS")
        m01 = persist.tile([128, NOFF, SQT], bf16, name="m01")
        ones = persist.tile([128, 1], bf16, name="ones")
        ident = persist.tile([128, 128], bf16, name="ident")

        nc.gpsimd.dma_start(ones[:], one_d[:])
        make_identity(nc, ident[:])
        # dummy broadcast: loads the gpsimd ucode overlay (~10us) off the
        # critical path — the first real one otherwise stalls t=0 normalize
        warm = persist.tile([128, 1], bf16, name="warm")
        nc.gpsimd.partition_broadcast(warm[:], ones[0:1, :])

        # ---------------- Phase 1: projections + RoPE + v ----------------
        with (
            tc.tile_pool(name="w1", bufs=1) as w1p,
            tc.tile_pool(name="xa", bufs=2) as xap,
            tc.tile_pool(name="raw", bufs=2) as rawp,
            tc.tile_pool(name="rope", bufs=2) as ropep,
            tc.tile_pool(name="ps1", bufs=1, space="PSUM") as ps1,
            tc.tile_pool(name="tps", bufs=2, space="PSUM") as tps,
        ):
            # wq and t0's x in 2-chunk piece TILES: cross-engine waits are
            # tile-granular, so small tiles let the first matmuls fire as
            # soon as their own 0.25MB lands instead of after the whole
            # tensor. All three rings split ~310GB/s early — keep the
            # first-15us traffic to exactly what the PE needs.
            wq_p8 = [w1p.tile([128, 2, GH * HD], bf16, name=f"wq{i}")
                     for i in range(8)]
            wk_sb = w1p.tile([128, NCH, HD], bf16)
            wv_sb = w1p.tile([128, NCH, HD], bf16)
            vT_tmp = w1p.tile([128, S], bf16)
            x0_p8 = [xap.tile([128, 2, SQT], bf16, tag=f"x0_{i}",
                              name=f"x0_{i}") for i in range(8)]
            for i in range(8):
                nc.scalar.dma_start(wq_p8[i][:], wq_v[:, 2 * i:2 * i + 2, :])
                nc.sync.dma_start(x0_p8[i][:], xT_v[:, 0, 2 * i:2 * i + 2, :])
            nc.scalar.dma_start(wk_sb[:], wk_v)
            nc.scalar.dma_start(wv_sb[:], wv_v)
            # rope/mask aux on the (otherwise idle) SWDGE ring
            nc.gpsimd.dma_start(cos2[:], cos_d[:])
            nc.gpsimd.dma_start(sinS[:], sin_d[:])
            nc.gpsimd.dma_start(m01[:], msk_d[:])

            def rope(raw, dst, t):
                """dst[:, t-tile] = rope(raw) in the [evens|odds] layout."""
                sl = np.s_[:, t * SQT:(t + 1) * SQT]
                tmp = ropep.tile([128, SQT], f32, tag="ropetmp", name="tmp")
                swp = ropep.tile([128, SQT], f32, tag="ropeswp", name="swp")
                nc.vector.tensor_mul(tmp[:], raw[:], cos2[sl])
                # swp[0:64] = odd*(-sin), swp[64:128] = even*(+sin); sinS is
                # stored [+sin | -sin] so each mul's two INPUTS share a base
                # partition (walrus requires that); only the output crosses.
                nc.vector.tensor_mul(swp[0:64, :], raw[64:128, :],
                                     sinS[sl][64:128, :])
                nc.vector.tensor_mul(swp[64:128, :], raw[0:64, :],
                                     sinS[sl][0:64, :])
                nc.vector.tensor_add(dst[sl], tmp[:], swp[:])

            for t in range(NSQ):
                ssl = np.s_[t * SQT:(t + 1) * SQT]
                q_ps = [ps1.tile([128, SQT], f32, tag=f"qps{h}", name=f"qps{h}")
                        for h in range(GH)]
                k_ps = ps1.tile([128, SQT], f32, tag="kps", name="k_ps")
                v_ps = ps1.tile([128, SQT], f32, tag="vps", name="v_ps")
                if t == 0:
                    def xc(c):
                        return x0_p8[c // 2][:, c % 2, :]
                else:
                    # two half-tiles per t: waits are tile-granular, so the
                    # PE starts on the first half while the second streams.
                    # t2 rides the scalar ring (idle after the weights) to
                    # spread the x load across both HWDGE rings.
                    eng = nc.scalar if t == 2 else nc.sync
                    xh = [xap.tile([128, NCH // 2, SQT], bf16, tag="xh",
                                   name="xh", bufs=4) for _ in range(2)]
                    eng.dma_start(xh[0][:], xT_v[:, t, 0:8, :])
                    eng.dma_start(xh[1][:], xT_v[:, t, 8:16, :])

                    def xc(c):
                        return xh[c // 8][:, c % 8, :]
                # all q matmuls first, then k/v: the PE stream is in-order,
                # and the (later-arriving) wk/wv DMAs must not stall it
                # while q chunks are ready
                for c in range(NCH):
                    st, sp = c == 0, c == NCH - 1
                    for h in range(GH):
                        nc.tensor.matmul(
                            q_ps[h][:],
                            wq_p8[c // 2][:, c % 2, h * HD:(h + 1) * HD],
                            xc(c), start=st, stop=sp)
                for c in range(NCH):
                    st, sp = c == 0, c == NCH - 1
                    nc.tensor.matmul(k_ps[:], wk_sb[:, c, :],
                                     xc(c), start=st, stop=sp)
                    nc.tensor.matmul(v_ps[:], wv_sb[:, c, :],
                                     xc(c), start=st, stop=sp)
                # psum -> sbuf copies split over ACT/DVE; q0/q1 drain FIRST
                # so the next t's leading matmuls get their banks back,
                # transposes before rope so the DVE queue doesn't block the
                # PE on freeing transpose psum slots
                qraws = []
                for h in range(GH):
                    qraw = rawp.tile([128, SQT], f32, tag=f"qraw{h}",
                                     name=f"qraw{h}")
                    qraws.append(qraw)
                nc.scalar.copy(qraws[0][:], q_ps[0][:])
                nc.vector.tensor_copy(qraws[1][:], q_ps[1][:])
                nc.scalar.copy(vT_tmp[:, ssl], v_ps[:])
                for j in range(NOFF * t, NOFF * (t + 1)):
                    t_ps = tps.tile([128, 128], bf16, tag="tps", name="t_ps")
                    nc.tensor.transpose(
                        t_ps[:], vT_tmp[:, j * 128:(j + 1) * 128], ident[:])
                    if j % 2 == 0:
                        nc.vector.tensor_copy(vn[:, j, :], t_ps[:])
                    else:
                        nc.scalar.copy(vn[:, j, :], t_ps[:])
                kraw = rawp.tile([128, SQT], f32, tag="kraw", name="kraw")
                nc.scalar.copy(kraw[:], k_ps[:])
                rope(kraw, kT, t)
                for h in range(GH):
                    if h == 2:
                        nc.scalar.copy(qraws[2][:], q_ps[2][:])
                    elif h == 3:
                        nc.vector.tensor_copy(qraws[3][:], q_ps[3][:])
                    rope(qraws[h], qT[h], t)

        # -------- Phase 2: attention (h-inner), then o-projection --------
        with (
            tc.tile_pool(name="wo", bufs=1) as wop,
            tc.tile_pool(name="ot", bufs=1) as otp,
            tc.tile_pool(name="ex", bufs=2) as exp_p,
            tc.tile_pool(name="nrm", bufs=2) as nrm_p,
            tc.tile_pool(name="osb", bufs=2) as osb_p,
        ):
            wo_sb = wop.tile([128, GH, D], bf16)
            nc.scalar.dma_start(wo_sb[:], wo_v)
            outT = [otp.tile([128, S], bf16, name=f"outT{h}")
                    for h in range(GH)]

            # o-projection chunks: one (m, jd) chunk = 4 accumulating
            # matmuls + a psum copy. Chunks are woven INTO the attention
            # j-loop as PE filler: attention alone is exp(ACT)-paced, and a
            # PE that micro-idles gets clocked down (p-state/HAM) to half
            # rate — the filler keeps it dense and at full clock.
            ob_map = {}
            ochunks = []        # ready (m, jd) chunks, global order
            oc_i = 0

            def o_chunk(ops_pool):
                nonlocal oc_i
                m, jd = ochunks[oc_i]
                oc_i += 1
                if jd == 0:
                    ob_map[m] = osb_p.tile([128, D], bf16, tag="ob",
                                           name="ob")
                dsl = np.s_[jd * SQT:(jd + 1) * SQT]
                o_ps = ops_pool.tile([128, SQT], f32, tag="o", name="o_ps")
                for h in range(GH):
                    nc.tensor.matmul(
                        o_ps[:], outT[h][:, m * 128:(m + 1) * 128],
                        wo_sb[:, h, dsl],
                        start=(h == 0), stop=(h == GH - 1))
                if (m + jd) % 2 == 0:
                    nc.vector.tensor_copy(ob_map[m][:, dsl], o_ps[:])
                else:
                    nc.scalar.copy(ob_map[m][:, dsl], o_ps[:])
                # store per-jd so the final rows drain while the PE still
                # computes (shrinks the end-of-kernel DMA tail)
                if (m + jd) % 2 == 0:
                    nc.sync.dma_start(out_v[:, m, dsl], ob_map[m][:, dsl])
                else:
                    nc.scalar.dma_start(out_v[:, m, dsl], ob_map[m][:, dsl])

            # t=1 first: its leading blocks are off-diagonal (no DVE mask in
            # the chain), so attention streams while DVE drains the phase-1
            # rope backlog; all-diagonal t=0 follows.
            with (
                tc.tile_pool(name="sps", bufs=2, space="PSUM") as sps_p,
                tc.tile_pool(name="avps", bufs=1, space="PSUM") as avp_p,
                tc.tile_pool(name="zps", bufs=1, space="PSUM") as zp_p,
                tc.tile_pool(name="opsi", bufs=1, space="PSUM") as ops_i,
            ):
                for t in [1, 0, 2, 3]:
                    qsl = np.s_[:, t * SQT:(t + 1) * SQT]
                    nblk = NOFF * (t + 1)
                    av_ps = [avp_p.tile([HD, SQT], f32, tag=f"av{h}",
                                        name=f"av{h}") for h in range(GH)]
                    z4 = zp_p.tile([128, SQT], f32, tag="z4", name="z4")
                    for j in range(nblk):
                        o = j - NOFF * t
                        lo = NARROW[o] if o >= 0 else 0
                        csl = np.s_[:, lo:SQT]
                        st, sp = j == 0, j == nblk - 1
                        # scores for all 4 heads first (s0,s1 | filler |
                        # s2,s3): by the time av_h streams, exp_h is done —
                        # the PE never blocks on the ACT engine
                        s_tiles = []
                        for h in range(GH):
                            s_ps = sps_p.tile([128, SQT], f32, tag="s",
                                              name="s_ps")
                            nc.tensor.matmul(
                                s_ps[csl], kT[:, j * 128:(j + 1) * 128],
                                qT[h][qsl][csl])
                            s_tiles.append(s_ps)
                            if h == 1 and oc_i < len(ochunks):
                                o_chunk(ops_i)
                        es = []
                        for h in range(GH):
                            e = exp_p.tile([128, SQT], bf16, tag=f"e{h}",
                                           name=f"e{h}")
                            nc.scalar.activation(e[csl], s_tiles[h][csl],
                                                 EXP, scale=SCALE)
                            if o >= 0:  # mask the causally-dead triangle
                                ma, mb = MASKW[o]
                                msl = np.s_[:, ma:mb]
                                nc.vector.tensor_mul(e[msl], e[msl],
                                                     m01[:, o, ma:mb])
                            es.append(e)
                        for h in range(GH):
                            nc.tensor.matmul(av_ps[h][csl], vn[:, j, :],
                                             es[h][csl], start=st, stop=sp)
                        # Z (softmax denominator) for all 4 heads: M=1
                        # matmuls col-tiled to distinct PE column groups so
                        # they overlap; head h's row lives at partition 32h
                        # (engine APs may only start at partitions 0/32/64/96)
                        for h in range(GH):
                            nc.tensor.matmul(
                                z4[32 * h:32 * h + 1, lo:SQT], ones[:],
                                es[h][csl], start=st, stop=sp,
                                tile_position=(0, 32 * h))
                    # free the AV psum banks fast: unnormalized copy on ACT
                    av_sb = []
                    for h in range(GH):
                        avo = nrm_p.tile([HD, SQT], f32, tag=f"avo{h}",
                                         name=f"avo{h}", bufs=1)
                        nc.scalar.copy(avo[:], av_ps[h][:])
                        av_sb.append(avo)
                    # unused zall rows memset to 1.0 so the batched
                    # reciprocal stays finite
                    zall = nrm_p.tile([128, SQT], f32, tag="zall", name="zall")
                    nc.gpsimd.memset(zall[:], 1.0)
                    for h in range(GH):
                        nc.vector.tensor_copy(zall[32 * h:32 * h + 1, :],
                                              z4[32 * h:32 * h + 1, :])
                    zrec = nrm_p.tile([128, SQT], f32, tag="zrec", name="zrec")
                    # ~18 significant bits — plenty above bf16 data, 5x
                    # faster than exact reciprocal; zall has no zeros/infs
                    nc.vector.reciprocal_approx_fast(zrec[:], zall[:])
                    for h in range(GH):
                        # stage to base partition 0: the HW broadcast ucode
                        # does not honor a nonzero AP base partition
                        z1 = nrm_p.tile([1, SQT], f32, tag="z1", name="z1")
                        nc.vector.tensor_copy(z1[:], zrec[32 * h:32 * h + 1, :])
                        zb = nrm_p.tile([128, SQT], f32, tag="zb", name="zb")
                        nc.gpsimd.partition_broadcast(zb[:], z1[:])
                        nc.vector.tensor_mul(outT[h][qsl], av_sb[h][:], zb[:])
                    # this tile's o-proj chunks become available as filler
                    # for the NEXT tile's attention
                    ochunks.extend([(m, jd) for m in
                                    range(NOFF * t, NOFF * (t + 1))
                                    for jd in range(D // SQT)])

            # remaining o-projection chunks back-to-back (deeper psum
            # rotation so copies never gate the PE)
            with tc.tile_pool(name="ops", bufs=4, space="PSUM") as ops_t:
                while oc_i < len(ochunks):
                    o_chunk(ops_t)

    nc.compile()
    return nc


_prog = None


def _host_inputs(x, wq, wk, wv, wo):
    """Per-core input maps (core c -> batch c//KV, kv-group c%KV)."""
    import ml_dtypes
    bf16 = ml_dtypes.bfloat16

    perm = np.concatenate([np.arange(0, HD, 2), np.arange(1, HD, 2)])
    wq_p = np.ascontiguousarray(
        wq.reshape(D, H, HD)[:, :, perm].reshape(D, H * HD))
    wk_p = np.ascontiguousarray(
        wk.reshape(D, KV, HD)[:, :, perm].reshape(D, KV * HD))

    inv_freq = 1.0 / (THETA ** (np.arange(0, HD, 2, dtype=np.float64) / HD))
    freqs = np.outer(np.arange(S, dtype=np.float64), inv_freq)   # [S, 64]
    cosT = np.cos(freqs).T.astype(np.float32)                    # [64, S]
    sinT = np.sin(freqs).T.astype(np.float32)
    cos2 = np.ascontiguousarray(np.concatenate([cosT, cosT], 0))
    sinS = np.ascontiguousarray(np.concatenate([sinT, -sinT], 0))

    sk = np.arange(128)[:, None]
    sq = np.arange(SQT)[None, :]
    m01 = np.stack([(sk <= sq - 128 * o).astype(np.float32)
                    for o in range(NOFF)], axis=1)               # [128,4,512]
    m01 = np.ascontiguousarray(m01).astype(bf16)

    def part_major(a):
        """[NCH*128, W] -> [128, NCH, W] (chunk-of-contraction per partition)."""
        w = a.shape[1]
        return np.ascontiguousarray(
            a.reshape(NCH, 128, w).transpose(1, 0, 2)).astype(bf16)

    in_maps = []
    for c in range(NCORES):
        b, g = c // KV, c % KV
        xr = np.ascontiguousarray(
            x[b].T.reshape(NCH, 128, NSQ, SQT).transpose(1, 2, 0, 3)
        ).astype(bf16)
        wog = np.ascontiguousarray(
            wo[g * GH * HD:(g + 1) * GH * HD, :].reshape(GH, 128, D)
            .transpose(1, 0, 2)).astype(bf16)
        in_maps.append({
            "xr": xr,
            "wqp": part_major(wq_p[:, g * GH * HD:(g + 1) * GH * HD]),
            "wkp": part_major(wk_p[:, g * HD:(g + 1) * HD]),
            "wvg": part_major(wv[:, g * HD:(g + 1) * HD]),
            "wog": wog,
            "cos2": cos2,
            "sinS": sinS,
            "m01": m01,
            "ones1": np.ones((128, 1), bf16),
        })
    return in_maps


def _numpy_reference(x, mask, wq, wk, wv, wo):
    """Pure-numpy fallback for inputs this kernel isn't specialized for."""
    b, s, _ = x.shape
    q = (x @ wq).reshape(b, s, H, HD)
    k = (x @ wk).reshape(b, s, KV, HD)
    v = (x @ wv).reshape(b, s, KV, HD)
    inv_freq = 1.0 / (THETA ** (np.arange(0, HD, 2, dtype=np.float32) / HD))
    t = np.arange(s, dtype=np.float32)
    freqs = np.outer(t, inv_freq)
    cos = np.cos(freqs)[:, None, :]
    sin = np.sin(freqs)[:, None, :]

    def rot(a):
        bb, ss, nh, hd = a.shape
        a = a.reshape(bb, ss, nh, hd // 2, 2)
        a0, a1 = a[..., 0], a[..., 1]
        out = np.stack([a0 * cos - a1 * sin, a0 * sin + a1 * cos], axis=-1)
        return out.reshape(bb, ss, nh, hd)

    q, k = rot(q), rot(k)
    rep = H // KV
    k = np.repeat(k, rep, axis=2)
    v = np.repeat(v, rep, axis=2)
    q, k, v = (a.transpose(0, 2, 1, 3) for a in (q, k, v))
    scores = np.einsum("bhqd,bhkd->bhqk", q, k) * SCALE + mask
    scores = scores - scores.max(axis=-1, keepdims=True)
    e = np.exp(scores)
    attn = e / e.sum(axis=-1, keepdims=True)
    out = np.einsum("bhqk,bhkd->bhqd", attn, v)
    out = out.transpose(0, 2, 1, 3).reshape(b, s, H * HD)
    return (out @ wo).astype(np.float32)


def kernel(x, mask, wq, wk, wv, wo):
    global _prog, last_results
    x = np.asarray(x, np.float32)
    mask = np.asarray(mask, np.float32)
    wq, wk, wv, wo = (np.asarray(a, np.float32) for a in (wq, wk, wv, wo))

    causal = np.where(np.tril(np.ones((S, S), bool)), 0.0, NEG).astype(np.float32)
    if (x.shape != (B, S, D) or mask.shape != (S, S)
            or not np.array_equal(mask, causal)):
        return _numpy_reference(x, mask, wq, wk, wv, wo)

    from concourse import bass_utils

    if _prog is None:
        _prog = _build_program()

    in_maps = _host_inputs(x, wq, wk, wv, wo)
    last_results = bass_utils.run_bass_kernel_spmd(
        _prog, in_maps, core_ids=list(range(NCORES)))
    # device layout [128, 16, D]: logical row = m*128 + p
    parts = [res["out"].astype(np.float32).transpose(1, 0, 2).reshape(S, D)
             for res in last_results.results]
    out = np.empty((B, S, D), np.float32)
    for b in range(B):
        out[b] = parts[KV * b] + parts[KV * b + 1] + parts[KV * b + 2] + parts[KV * b + 3]
    return out


# revision 21
# speedup vs baseline: 1.0389x; 1.0145x over previous
"""GQA causal attention (B=2,S=2048,D=2048,H=16,KV=4,HD=128) on 8 TRN2 NeuronCores.

Sharding: core c handles (batch b=c//4, kv-group g=c%4) — exactly 8 shards.
Each core computes q/k/v projections for its group's 4 query heads + 1 kv head,
RoPE, causal attention (512-wide q tiles, skipping fully-masked k blocks),
and a partial o-projection over its heads' slice of wo. Host sums the 4
group-partials per batch.

v2 (this file) vs the f32r baseline:
  - whole matmul datapath in bf16 (x, wq/wk/wv/wo, qT/kT/vn, e, outT, out).
    PSUM accumulation stays f32; rope/softmax-normalize math stays f32.
    Halves the input DMA (x was the phase-1 bottleneck: it rode a ~100GB/s
    SWDGE queue while total SDMA is ~350GB/s shared) and lifts the f32r
    N>=256 restriction so diagonal score blocks narrow to their live range.
  - all bulk DMA on the two HWDGE rings (sync + scalar); gpsimd SWDGE only
    for tiny loads. x on sync, weights on scalar, first tile split fine.
  - phase 2 restructured h-inner: per (t, j) the 4 heads' scores/exp/AV run
    back-to-back and the 4 Z (softmax denominator) matmuls are col-tiled
    (M=1 at tile_position (0,32h)) into one PSUM tile so they overlap on
    distinct PE column groups instead of costing a full third pass.
  - o-projection moved after the attention loop (attention needs all 8 PSUM
    banks: 3 score + 4 AV + 1 Z); its rows are ordered so earlier q-tiles'
    output flows while the last tile's normalize chain drains.

Device layouts are all "transposed" ([feature, seq]) so no on-device
transposes of activations are needed; head-dim is PERMUTED to [evens|odds]
(folded into wq/wk columns host-side) so RoPE is two partition-aligned
half-tile multiplies.
"""

import numpy as np

B, S, D = 2, 2048, 2048
H, KV, HD = 16, 4, 128
GH = H // KV            # query heads per kv group (per core)
NCORES = 8
THETA = 10000.0
NEG = -1e9
SQT = 512               # q seq tile width
NSQ = S // SQT          # 4
NKB = S // 128          # 16 k blocks
NCH = D // 128          # 16 contraction chunks
NOFF = SQT // 128       # 4 diagonal offsets

SCALE = float(HD) ** -0.5

# Diagonal-region block at offset o is causally dead below sq_local = 128*o;
# bf16 matmuls run full rate at any N, so narrow exactly to the live range.
NARROW = [0, 128, 256, 384]
# The 128-wide triangular column range needing the 0/1 mask multiply.
MASKW = [(0, 128), (128, 256), (256, 384), (384, 512)]

# Exposed for the dev harness (test.py) to read profiling results.
last_results = None


def _build_program():
    from contextlib import ExitStack

    import concourse.tile as tile
    from concourse import bacc, mybir
    from concourse.masks import make_identity

    f32 = mybir.dt.float32
    bf16 = mybir.dt.bfloat16
    EXP = mybir.ActivationFunctionType.Exp

    nc = bacc.Bacc("TRN2", target_bir_lowering=False, debug=False,
                   num_devices=NCORES)

    # all bulk tensors are pre-rearranged on the host so every DMA is
    # contiguous per partition
    xT_d = nc.dram_tensor("xr", [128, NSQ, NCH, SQT], bf16, kind="ExternalInput")
    wq_d = nc.dram_tensor("wqp", [128, NCH, GH * HD], bf16, kind="ExternalInput")
    wk_d = nc.dram_tensor("wkp", [128, NCH, HD], bf16, kind="ExternalInput")
    wv_d = nc.dram_tensor("wvg", [128, NCH, HD], bf16, kind="ExternalInput")
    wo_d = nc.dram_tensor("wog", [128, GH, D], bf16, kind="ExternalInput")
    cos_d = nc.dram_tensor("cos2", [HD, S], bf16, kind="ExternalInput")
    sin_d = nc.dram_tensor("sinS", [HD, S], bf16, kind="ExternalInput")
    msk_d = nc.dram_tensor("m01", [128, NOFF, SQT], bf16, kind="ExternalInput")
    one_d = nc.dram_tensor("ones1", [128, 1], bf16, kind="ExternalInput")
    out_d = nc.dram_tensor("out", [128, S // 128, D], bf16, kind="ExternalOutput")

    xT_v = xT_d.ap()        # [128, NSQ, NCH, SQT]
    wq_v = wq_d.ap()
    wk_v = wk_d.ap()
    wv_v = wv_d.ap()
    wo_v = wo_d.ap()
    out_v = out_d.ap()      # [128, 16, 2048]; host untangles (m p) rows

    with tile.TileContext(nc) as tc, ExitStack() as ctx:
        persist = ctx.enter_context(tc.tile_pool(name="persist", bufs=1))

        qT = [persist.tile([128, S], bf16, name=f"qT{h}") for h in range(GH)]
        kT = persist.tile([128, S], bf16, name="kT")
        vn = persist.tile([128, NKB, HD], bf16, name="vn")
        cos2 = persist.tile([128, S], bf16, name="cos2")
        sinS = persist.tile([128, S], bf16, name="sinS")
        m01 = persist.tile([128, NOFF, SQT], bf16, name="m01")
        ones = persist.tile([128, 1], bf16, name="ones")
        ident = persist.tile([128, 128], bf16, name="ident")

        nc.gpsimd.dma_start(ones[:], one_d[:])
        make_identity(nc, ident[:])
        # dummy broadcast: loads the gpsimd ucode overlay (~10us) off the
        # critical path — the first real one otherwise stalls t=0 normalize
        warm = persist.tile([128, 1], bf16, name="warm")
        nc.gpsimd.partition_broadcast(warm[:], ones[0:1, :])

        # ---------------- Phase 1: projections + RoPE + v ----------------
        with (
            tc.tile_pool(name="w1", bufs=1) as w1p,
            tc.tile_pool(name="xa", bufs=2) as xap,
            tc.tile_pool(name="raw", bufs=2) as rawp,
            tc.tile_pool(name="rope", bufs=2) as ropep,
            tc.tile_pool(name="ps1", bufs=1, space="PSUM") as ps1,
            tc.tile_pool(name="tps", bufs=2, space="PSUM") as tps,
        ):
            # wq and t0's x in 2-chunk piece TILES: cross-engine waits are
            # tile-granular, so small tiles let the first matmuls fire as
            # soon as their own 0.25MB lands instead of after the whole
            # tensor. All three rings split ~310GB/s early — keep the
            # first-15us traffic to exactly what the PE needs.
            wq_p8 = [w1p.tile([128, 2, GH * HD], bf16, name=f"wq{i}")
                     for i in range(8)]
            wk_sb = w1p.tile([128, NCH, HD], bf16)
            wv_sb = w1p.tile([128, NCH, HD], bf16)
            vT_tmp = w1p.tile([128, S], bf16)
            x0_p8 = [xap.tile([128, 2, SQT], bf16, tag=f"x0_{i}",
                              name=f"x0_{i}") for i in range(8)]
            for i in range(8):
                nc.scalar.dma_start(wq_p8[i][:], wq_v[:, 2 * i:2 * i + 2, :])
                nc.sync.dma_start(x0_p8[i][:], xT_v[:, 0, 2 * i:2 * i + 2, :])
            nc.scalar.dma_start(wk_sb[:], wk_v)
            nc.scalar.dma_start(wv_sb[:], wv_v)
            # rope aux on the (otherwise idle) SWDGE ring; m01 is deferred
            # to the phase-2 prologue (it's only read by the mask muls)
            nc.gpsimd.dma_start(cos2[:], cos_d[:])
            nc.gpsimd.dma_start(sinS[:], sin_d[:])

            def rope(raw, dst, t):
                """dst[:, t-tile] = rope(raw) in the [evens|odds] layout."""
                sl = np.s_[:, t * SQT:(t + 1) * SQT]
                tmp = ropep.tile([128, SQT], f32, tag="ropetmp", name="tmp")
                swp = ropep.tile([128, SQT], f32, tag="ropeswp", name="swp")
                nc.vector.tensor_mul(tmp[:], raw[:], cos2[sl])
                # swp[0:64] = odd*(-sin), swp[64:128] = even*(+sin); sinS is
                # stored [+sin | -sin] so each mul's two INPUTS share a base
                # partition (walrus requires that); only the output crosses.
                nc.vector.tensor_mul(swp[0:64, :], raw[64:128, :],
                                     sinS[sl][64:128, :])
                nc.vector.tensor_mul(swp[64:128, :], raw[0:64, :],
                                     sinS[sl][0:64, :])
                nc.vector.tensor_add(dst[sl], tmp[:], swp[:])

            for t in range(NSQ):
                ssl = np.s_[t * SQT:(t + 1) * SQT]
                q_ps = [ps1.tile([128, SQT], f32, tag=f"qps{h}", name=f"qps{h}")
                        for h in range(GH)]
                k_ps = ps1.tile([128, SQT], f32, tag="kps", name="k_ps")
                v_ps = ps1.tile([128, SQT], f32, tag="vps", name="v_ps")
                if t == 0:
                    def xc(c):
                        return x0_p8[c // 2][:, c % 2, :]
                else:
                    # two half-tiles per t: waits are tile-granular, so the
                    # PE starts on the first half while the second streams.
                    # t2 rides the scalar ring (idle after the weights) to
                    # spread the x load across both HWDGE rings.
                    xh = [xap.tile([128, NCH // 2, SQT], bf16, tag="xh",
                                   name="xh", bufs=4) for _ in range(2)]
                    eng_a = nc.scalar if t == 2 else nc.sync
                    eng_b = nc.sync if t == 1 else nc.scalar
                    eng_a.dma_start(xh[0][:], xT_v[:, t, 0:8, :])
                    eng_b.dma_start(xh[1][:], xT_v[:, t, 8:16, :])

                    def xc(c):
                        return xh[c // 8][:, c % 8, :]
                # all q matmuls first, then k/v: the PE stream is in-order,
                # and the (later-arriving) wk/wv DMAs must not stall it
                # while q chunks are ready
                for c in range(NCH):
                    st, sp = c == 0, c == NCH - 1
                    for h in range(GH):
                        nc.tensor.matmul(
                            q_ps[h][:],
                            wq_p8[c // 2][:, c % 2, h * HD:(h + 1) * HD],
                            xc(c), start=st, stop=sp)
                for c in range(NCH):
                    st, sp = c == 0, c == NCH - 1
                    nc.tensor.matmul(k_ps[:], wk_sb[:, c, :],
                                     xc(c), start=st, stop=sp)
                    nc.tensor.matmul(v_ps[:], wv_sb[:, c, :],
                                     xc(c), start=st, stop=sp)
                # psum -> sbuf copies split over ACT/DVE; q0/q1 drain FIRST
                # so the next t's leading matmuls get their banks back,
                # transposes before rope so the DVE queue doesn't block the
                # PE on freeing transpose psum slots
                qraws = []
                for h in range(GH):
                    qraw = rawp.tile([128, SQT], f32, tag=f"qraw{h}",
                                     name=f"qraw{h}")
                    qraws.append(qraw)
                nc.scalar.copy(qraws[0][:], q_ps[0][:])
                nc.vector.tensor_copy(qraws[1][:], q_ps[1][:])
                nc.scalar.copy(vT_tmp[:, ssl], v_ps[:])
                for j in range(NOFF * t, NOFF * (t + 1)):
                    t_ps = tps.tile([128, 128], bf16, tag="tps", name="t_ps")
                    nc.tensor.transpose(
                        t_ps[:], vT_tmp[:, j * 128:(j + 1) * 128], ident[:])
                    if j % 2 == 0:
                        nc.vector.tensor_copy(vn[:, j, :], t_ps[:])
                    else:
                        nc.scalar.copy(vn[:, j, :], t_ps[:])
                kraw = rawp.tile([128, SQT], f32, tag="kraw", name="kraw")
                nc.scalar.copy(kraw[:], k_ps[:])
                rope(kraw, kT, t)
                for h in range(GH):
                    if h == 2:
                        nc.scalar.copy(qraws[2][:], q_ps[2][:])
                    elif h == 3:
                        nc.vector.tensor_copy(qraws[3][:], q_ps[3][:])
                    rope(qraws[h], qT[h], t)

        # -------- Phase 2: attention (h-inner), then o-projection --------
        with (
            tc.tile_pool(name="wo", bufs=1) as wop,
            tc.tile_pool(name="ot", bufs=1) as otp,
            tc.tile_pool(name="ex", bufs=2) as exp_p,
            tc.tile_pool(name="nrm", bufs=2) as nrm_p,
            tc.tile_pool(name="osb", bufs=2) as osb_p,
        ):
            wo_sb = wop.tile([128, GH, D], bf16)
            nc.gpsimd.dma_start(m01[:], msk_d[:])
            nc.scalar.dma_start(wo_sb[:], wo_v)
            outT = [otp.tile([128, S], bf16, name=f"outT{h}")
                    for h in range(GH)]

            # o-projection chunks: one (m, jd) chunk = 4 accumulating
            # matmuls + a psum copy. Chunks are woven INTO the attention
            # j-loop as PE filler: attention alone is exp(ACT)-paced, and a
            # PE that micro-idles gets clocked down (p-state/HAM) to half
            # rate — the filler keeps it dense and at full clock.
            ob_map = {}
            ochunks = []        # ready (m, jd) chunks, global order
            oc_i = 0

            def o_chunk(ops_pool):
                nonlocal oc_i
                m, jd = ochunks[oc_i]
                oc_i += 1
                if jd == 0:
                    ob_map[m] = osb_p.tile([128, D], bf16, tag="ob",
                                           name="ob")
                dsl = np.s_[jd * SQT:(jd + 1) * SQT]
                o_ps = ops_pool.tile([128, SQT], f32, tag="o", name="o_ps")
                for h in range(GH):
                    nc.tensor.matmul(
                        o_ps[:], outT[h][:, m * 128:(m + 1) * 128],
                        wo_sb[:, h, dsl],
                        start=(h == 0), stop=(h == GH - 1))
                if (m + jd) % 2 == 0:
                    nc.vector.tensor_copy(ob_map[m][:, dsl], o_ps[:])
                else:
                    nc.scalar.copy(ob_map[m][:, dsl], o_ps[:])
                # store per-jd so the final rows drain while the PE still
                # computes (shrinks the end-of-kernel DMA tail)
                if (m + jd) % 2 == 0:
                    nc.sync.dma_start(out_v[:, m, dsl], ob_map[m][:, dsl])
                else:
                    nc.scalar.dma_start(out_v[:, m, dsl], ob_map[m][:, dsl])

            # t=1 first: its leading blocks are off-diagonal (no DVE mask in
            # the chain), so attention streams while DVE drains the phase-1
            # rope backlog; all-diagonal t=0 follows.
            with (
                tc.tile_pool(name="sps", bufs=2, space="PSUM") as sps_p,
                tc.tile_pool(name="avps", bufs=1, space="PSUM") as avp_p,
                tc.tile_pool(name="zps", bufs=1, space="PSUM") as zp_p,
                tc.tile_pool(name="opsi", bufs=1, space="PSUM") as ops_i,
            ):
                for t in [1, 0, 2, 3]:
                    qsl = np.s_[:, t * SQT:(t + 1) * SQT]
                    nblk = NOFF * (t + 1)
                    av_ps = [avp_p.tile([HD, SQT], f32, tag=f"av{h}",
                                        name=f"av{h}") for h in range(GH)]
                    z4 = zp_p.tile([128, SQT], f32, tag="z4", name="z4")
                    for j in range(nblk):
                        o = j - NOFF * t
                        lo = NARROW[o] if o >= 0 else 0
                        csl = np.s_[:, lo:SQT]
                        st, sp = j == 0, j == nblk - 1
                        # scores for all 4 heads first (s0,s1 | filler |
                        # s2,s3): by the time av_h streams, exp_h is done —
                        # the PE never blocks on the ACT engine
                        s_tiles = []
                        for h in range(GH):
                            s_ps = sps_p.tile([128, SQT], f32, tag="s",
                                              name="s_ps")
                            nc.tensor.matmul(
                                s_ps[csl], kT[:, j * 128:(j + 1) * 128],
                                qT[h][qsl][csl])
                            s_tiles.append(s_ps)
                            if h == 1 and oc_i < len(ochunks):
                                o_chunk(ops_i)
                        es = []
                        for h in range(GH):
                            e = exp_p.tile([128, SQT], bf16, tag=f"e{h}",
                                           name=f"e{h}")
                            nc.scalar.activation(e[csl], s_tiles[h][csl],
                                                 EXP, scale=SCALE)
                            if o >= 0:  # mask the causally-dead triangle
                                ma, mb = MASKW[o]
                                msl = np.s_[:, ma:mb]
                                nc.vector.tensor_mul(e[msl], e[msl],
                                                     m01[:, o, ma:mb])
                            es.append(e)
                        for h in range(GH):
                            nc.tensor.matmul(av_ps[h][csl], vn[:, j, :],
                                             es[h][csl], start=st, stop=sp)
                        # Z (softmax denominator) for all 4 heads: M=1
                        # matmuls col-tiled to distinct PE column groups so
                        # they overlap; head h's row lives at partition 32h
                        # (engine APs may only start at partitions 0/32/64/96)
                        for h in range(GH):
                            nc.tensor.matmul(
                                z4[32 * h:32 * h + 1, lo:SQT], ones[:],
                                es[h][csl], start=st, stop=sp,
                                tile_position=(0, 32 * h))
                    # free the AV psum banks fast: unnormalized copy on ACT
                    av_sb = []
                    for h in range(GH):
                        avo = nrm_p.tile([HD, SQT], f32, tag=f"avo{h}",
                                         name=f"avo{h}", bufs=1)
                        nc.scalar.copy(avo[:], av_ps[h][:])
                        av_sb.append(avo)
                    # unused zall rows memset to 1.0 so the batched
                    # reciprocal stays finite
                    zall = nrm_p.tile([128, SQT], f32, tag="zall", name="zall")
                    nc.gpsimd.memset(zall[:], 1.0)
                    for h in range(GH):
                        nc.vector.tensor_copy(zall[32 * h:32 * h + 1, :],
                                              z4[32 * h:32 * h + 1, :])
                    zrec = nrm_p.tile([128, SQT], f32, tag="zrec", name="zrec")
                    # ~18 significant bits — plenty above bf16 data, 5x
                    # faster than exact reciprocal; zall has no zeros/infs
                    nc.vector.reciprocal_approx_fast(zrec[:], zall[:])
                    for h in range(GH):
                        # stage to base partition 0: the HW broadcast ucode
                        # does not honor a nonzero AP base partition
                        z1 = nrm_p.tile([1, SQT], f32, tag="z1", name="z1")
                        nc.vector.tensor_copy(z1[:], zrec[32 * h:32 * h + 1, :])
                        zb = nrm_p.tile([128, SQT], f32, tag="zb", name="zb")
                        nc.gpsimd.partition_broadcast(zb[:], z1[:])
                        nc.vector.tensor_mul(outT[h][qsl], av_sb[h][:], zb[:])
                    # this tile's o-proj chunks become available as filler
                    # for the NEXT tile's attention
                    ochunks.extend([(m, jd) for m in
                                    range(NOFF * t, NOFF * (t + 1))
                                    for jd in range(D // SQT)])

            # remaining o-projection chunks back-to-back (deeper psum
            # rotation so copies never gate the PE)
            with tc.tile_pool(name="ops", bufs=4, space="PSUM") as ops_t:
                while oc_i < len(ochunks):
                    o_chunk(ops_t)

    nc.compile()
    return nc


_prog = None


def _host_inputs(x, wq, wk, wv, wo):
    """Per-core input maps (core c -> batch c//KV, kv-group c%KV)."""
    import ml_dtypes
    bf16 = ml_dtypes.bfloat16

    perm = np.concatenate([np.arange(0, HD, 2), np.arange(1, HD, 2)])
    wq_p = np.ascontiguousarray(
        wq.reshape(D, H, HD)[:, :, perm].reshape(D, H * HD))
    wk_p = np.ascontiguousarray(
        wk.reshape(D, KV, HD)[:, :, perm].reshape(D, KV * HD))

    inv_freq = 1.0 / (THETA ** (np.arange(0, HD, 2, dtype=np.float64) / HD))
    freqs = np.outer(np.arange(S, dtype=np.float64), inv_freq)   # [S, 64]
    cosT = np.cos(freqs).T.astype(np.float32)                    # [64, S]
    sinT = np.sin(freqs).T.astype(np.float32)
    cos2 = np.ascontiguousarray(np.concatenate([cosT, cosT], 0)).astype(bf16)
    sinS = np.ascontiguousarray(np.concatenate([sinT, -sinT], 0)).astype(bf16)

    sk = np.arange(128)[:, None]
    sq = np.arange(SQT)[None, :]
    m01 = np.stack([(sk <= sq - 128 * o).astype(np.float32)
                    for o in range(NOFF)], axis=1)               # [128,4,512]
    m01 = np.ascontiguousarray(m01).astype(bf16)

    def part_major(a):
        """[NCH*128, W] -> [128, NCH, W] (chunk-of-contraction per partition)."""
        w = a.shape[1]
        return np.ascontiguousarray(
            a.reshape(NCH, 128, w).transpose(1, 0, 2)).astype(bf16)

    in_maps = []
    for c in range(NCORES):
        b, g = c // KV, c % KV
        xr = np.ascontiguousarray(
            x[b].T.reshape(NCH, 128, NSQ, SQT).transpose(1, 2, 0, 3)
        ).astype(bf16)
        wog = np.ascontiguousarray(
            wo[g * GH * HD:(g + 1) * GH * HD, :].reshape(GH, 128, D)
            .transpose(1, 0, 2)).astype(bf16)
        in_maps.append({
            "xr": xr,
            "wqp": part_major(wq_p[:, g * GH * HD:(g + 1) * GH * HD]),
            "wkp": part_major(wk_p[:, g * HD:(g + 1) * HD]),
            "wvg": part_major(wv[:, g * HD:(g + 1) * HD]),
            "wog": wog,
            "cos2": cos2,
            "sinS": sinS,
            "m01": m01,
            "ones1": np.ones((128, 1), bf16),
        })
    return in_maps


def _numpy_reference(x, mask, wq, wk, wv, wo):
    """Pure-numpy fallback for inputs this kernel isn't specialized for."""
    b, s, _ = x.shape
    q = (x @ wq).reshape(b, s, H, HD)
    k = (x @ wk).reshape(b, s, KV, HD)
    v = (x @ wv).reshape(b, s, KV, HD)
    inv_freq = 1.0 / (THETA ** (np.arange(0, HD, 2, dtype=np.float32) / HD))
    t = np.arange(s, dtype=np.float32)
    freqs = np.outer(t, inv_freq)
    cos = np.cos(freqs)[:, None, :]
    sin = np.sin(freqs)[:, None, :]

    def rot(a):
        bb, ss, nh, hd = a.shape
        a = a.reshape(bb, ss, nh, hd // 2, 2)
        a0, a1 = a[..., 0], a[..., 1]
        out = np.stack([a0 * cos - a1 * sin, a0 * sin + a1 * cos], axis=-1)
        return out.reshape(bb, ss, nh, hd)

    q, k = rot(q), rot(k)
    rep = H // KV
    k = np.repeat(k, rep, axis=2)
    v = np.repeat(v, rep, axis=2)
    q, k, v = (a.transpose(0, 2, 1, 3) for a in (q, k, v))
    scores = np.einsum("bhqd,bhkd->bhqk", q, k) * SCALE + mask
    scores = scores - scores.max(axis=-1, keepdims=True)
    e = np.exp(scores)
    attn = e / e.sum(axis=-1, keepdims=True)
    out = np.einsum("bhqk,bhkd->bhqd", attn, v)
    out = out.transpose(0, 2, 1, 3).reshape(b, s, H * HD)
    return (out @ wo).astype(np.float32)


def kernel(x, mask, wq, wk, wv, wo):
    global _prog, last_results
    x = np.asarray(x, np.float32)
    mask = np.asarray(mask, np.float32)
    wq, wk, wv, wo = (np.asarray(a, np.float32) for a in (wq, wk, wv, wo))

    causal = np.where(np.tril(np.ones((S, S), bool)), 0.0, NEG).astype(np.float32)
    if (x.shape != (B, S, D) or mask.shape != (S, S)
            or not np.array_equal(mask, causal)):
        return _numpy_reference(x, mask, wq, wk, wv, wo)

    from concourse import bass_utils

    if _prog is None:
        _prog = _build_program()

    in_maps = _host_inputs(x, wq, wk, wv, wo)
    last_results = bass_utils.run_bass_kernel_spmd(
        _prog, in_maps, core_ids=list(range(NCORES)))
    # device layout [128, 16, D]: logical row = m*128 + p
    parts = [res["out"].astype(np.float32).transpose(1, 0, 2).reshape(S, D)
             for res in last_results.results]
    out = np.empty((B, S, D), np.float32)
    for b in range(B):
        out[b] = parts[KV * b] + parts[KV * b + 1] + parts[KV * b + 2] + parts[KV * b + 3]
    return out
